# revision 20
# baseline (speedup 1.0000x reference)
"""Trainium2 Bass kernel for a GNN message-passing layer.

Reference computation (per graph):
    src,dst = edge_indices
    h   = gelu(concat(x[src], x[dst], e) @ W1m + b1m)          # [E, H]
    msg = h @ W2m + b2m                                        # [E, H]
    agg = segment_sum(msg, dst)                                # [N, H]
    u   = gelu(concat(x, agg) @ W1u + b1u)                     # [N, H]
    out = u @ W2u + b2u                                        # [N, D]

Device strategy (8 cores = 2 graphs x 4 dst-ranges):
  - By linearity, W2m is applied AFTER aggregation: agg = segsum(h) @ W2m + deg*b2m.
  - W1m splits into per-node projections Psrc = x@W1m[:D] (stored fp8 in a DRAM
    table, row-gathered per edge) and Pdst = x@W1m[D:2D].
  - Edges are bucketed by 64-node destination block.  Per chunk of 128 edges the
    dense part of the gelu input is ONE matmul: lhsT = [onehot64(dst_rel) ;
    edge_feat] (128 rows), rhs = W_ext(blk) = [Pdst_blk + b1m ; W1m_edge] so the
    destination projection, edge projection and bias come out fused in PSUM.
  - h = gelu(Psrc[src] + PSUM), token-major [128e, H] tiles.
  - Scatter-add via one-hot matmul: sel[e,n] = (dst_rel[e]==n); per node block
    aggT[h,n] += h_chunk.T @ sel accumulated in PSUM -> feature-major aggregates.
  - Host pre-sorts each graph's edges by destination block (64 nodes/block,
    40 blocks/core) and pads each block's edge count to a multiple of 128.
"""

import os
import sys

sys.path.insert(0, "/opt/trn_rl_repo")

import numpy as np
import ml_dtypes

import concourse.bacc as bacc
import concourse.mybir as mybir
import concourse.tile as tile
from concourse.bass_utils import run_bass_kernel_spmd

BF16 = ml_dtypes.bfloat16
FP8 = ml_dtypes.float8_e4m3fn

B, N, E = 2, 10000, 160000
D, F, H = 128, 64, 256
NCORES = 8
CPG = NCORES // B          # cores per graph = 4
NBLK = 40                  # node blocks per core
BLK = 64                   # nodes per block
NSLICE = NBLK * BLK        # 2560 nodes per core
NPAD = CPG * NSLICE        # 10240 padded nodes per graph

f32 = mybir.dt.float32
bf16 = mybir.dt.bfloat16
fp8 = mybir.dt.float8e4
i16 = mybir.dt.int16

_BUILD_CACHE = {}


def _build(k0, k1, vcnt):
    """Build the SPMD single-core program.

    k0/k1 = 128-edge chunks per node block whose gather source is the
    low/high half of the Psrc table (so gathers can start after only half
    the table is built)."""
    k_blk = k0 + k1
    nchunk = NBLK * k_blk
    ecap = nchunk * 128

    nc = bacc.Bacc(None, num_swdge_queues=4)

    # ---- external inputs (per-core) ----
    nft = nc.dram_tensor("nft", [D, NPAD], bf16, kind="ExternalInput")
    nfs = nc.dram_tensor("nfs", [D, NSLICE], bf16, kind="ExternalInput")
    ehot = nc.dram_tensor("ehot", [128, ecap], fp8, kind="ExternalInput")
    selb = nc.dram_tensor("selb", [128, nchunk * BLK], fp8, kind="ExternalInput")
    gidx = nc.dram_tensor("gidx", [128, ecap // 16], i16, kind="ExternalInput")
    degrow = nc.dram_tensor("degrow", [1, NSLICE], bf16, kind="ExternalInput")
    w1e = nc.dram_tensor("w1e", [F, H], bf16, kind="ExternalInput")
    b1mr = nc.dram_tensor("b1mr", [1, H], bf16, kind="ExternalInput")
    wsrc = nc.dram_tensor("wsrc", [D, H], bf16, kind="ExternalInput")
    wdst = nc.dram_tensor("wdst", [D, H], bf16, kind="ExternalInput")
    # weight k-chunks packed side-by-side: [128, nchunks*cols]
    w2m = nc.dram_tensor("w2m", [128, 2 * H], bf16, kind="ExternalInput")
    b2mr = nc.dram_tensor("b2mr", [1, H], bf16, kind="ExternalInput")
    w1u = nc.dram_tensor("w1u", [128, 3 * H], bf16, kind="ExternalInput")
    b1uc = nc.dram_tensor("b1uc", [128, 2], f32, kind="ExternalInput")
    w2u = nc.dram_tensor("w2u", [128, 2 * D], bf16, kind="ExternalInput")
    b2ur = nc.dram_tensor("b2ur", [1, D], bf16, kind="ExternalInput")
    onesr = nc.dram_tensor("onesr", [1, 128], bf16, kind="ExternalInput")

    out = nc.dram_tensor("out", [NSLICE, D], f32, kind="ExternalOutput")

    with tile.TileContext(nc) as tc:
        with (
            tc.tile_pool(name="const", bufs=1) as cpool,
            tc.tile_pool(name="dram", bufs=1, space="DRAM") as dpool,
            tc.tile_pool(name="eftp", bufs=3) as eftp,
            tc.tile_pool(name="gath", bufs=4) as gpool,
            tc.tile_pool(name="sel", bufs=2) as selp,
            tc.tile_pool(name="hwork", bufs=6) as hp,
            tc.tile_pool(name="cp", bufs=4) as cp,
            tc.tile_pool(name="psA", bufs=2, space="PSUM") as psA,
            tc.tile_pool(name="psW", bufs=1, space="PSUM") as psW,
            tc.tile_pool(name="psS", bufs=1, space="PSUM") as psS,
            tc.tile_pool(name="agg", bufs=2, space="PSUM") as psG,
        ):
            # ---- load constants / persistent tensors into SBUF ----
            def load(dram_t, shape, dtype):
                t = cpool.tile(shape, dtype, tag=dram_t.name)
                nc.sync.dma_start(out=t[:], in_=dram_t[:])
                return t

            nft_s = cpool.tile([D, NPAD], bf16, tag="nft")
            NQ = NPAD // 4
            for q in range(4):
                nc.sync.dma_start(out=nft_s[:, q * NQ:(q + 1) * NQ],
                                  in_=nft[:, q * NQ:(q + 1) * NQ])
            nfs_s = load(nfs, [D, NSLICE], bf16)
            gidx_s = load(gidx, [128, ecap // 16], i16)
            degrow_s = load(degrow, [1, NSLICE], bf16)
            w1e_s = load(w1e, [F, H], bf16)
            b1mr_s = load(b1mr, [1, H], bf16)
            wsrc_s = load(wsrc, [D, H], bf16)
            wdst_s = load(wdst, [D, H], bf16)
            w2m_s = load(w2m, [128, 2 * H], bf16)
            b2mr_s = load(b2mr, [1, H], bf16)
            w1u_s = load(w1u, [128, 3 * H], bf16)
            b1uc_s = load(b1uc, [128, 2], f32)
            w2u_s = load(w2u, [128, 2 * D], bf16)
            b2ur_s = load(b2ur, [1, D], bf16)
            onesr_s = load(onesr, [1, 128], bf16)

            # Psrc table in DRAM (gather source, fp8, two halves so edge
            # gathers overlap the tail of table construction); W_ext in SBUF
            pall0 = dpool.tile([NPAD // 2, H], fp8)
            pall1 = dpool.tile([NPAD // 2, H], fp8)
            wext = cpool.tile([128, NBLK * H], bf16, tag="wext")

            # pre-zeroed gather tiles (pad slots are skipped by the gather and
            # must hold finite bytes for the downstream add/gelu/scatter)
            gts_pre = []
            for i in range(4):
                gtp = gpool.tile([128, k0 + k1, H], fp8, tag="gath",
                                 name=f"gtpre{i}")
                nc.gpsimd.memset(gtp[:], 0.0)
                gts_pre.append(gtp)

            # ---- stage 1: Psrc gather table first (gathers wait on it) ----
            PSB = 8  # table blocks staged per DMA write
            for nb0 in range(0, NPAD // 128, PSB):
                stg = cp.tile([128, PSB, H], fp8, tag="stg")
                for j in range(PSB):
                    nb = nb0 + j
                    ps = psA.tile([128, H], f32, tag="psA")
                    nc.tensor.matmul(
                        out=ps[:], lhsT=nft_s[:, nb * 128:(nb + 1) * 128],
                        rhs=wsrc_s[:], start=True, stop=True,
                    )
                    if nb % 2 == 0:
                        nc.vector.tensor_copy(out=stg[:, j, :], in_=ps[:])
                    else:
                        nc.scalar.copy(out=stg[:, j, :], in_=ps[:])
                half, rb = divmod(nb0, NPAD // 256)
                pall_h = pall0 if half == 0 else pall1
                nc.sync.dma_start(
                    out=pall_h[rb * 128:(rb + PSB) * 128, :]
                        .rearrange("(a p) h -> p a h", p=128),
                    in_=stg[:],
                )

            # per-512-node-group accumulators (feature-major, bf16)
            NG5 = NSLICE // 512
            BPG = 512 // BLK   # blocks per group = 8
            aggT = [[cpool.tile([128, 512], bf16, tag=f"aggT{o}_{g}",
                                name=f"aggT{o}_{g}") for g in range(NG5)]
                    for o in range(2)]
            agfT = [[cpool.tile([128, 512], bf16, tag=f"agfT{o}_{g}",
                                name=f"agfT{o}_{g}") for g in range(NG5)]
                    for o in range(2)]
            uT = [[cpool.tile([128, 512], bf16, tag=f"u{o}_{g}",
                              name=f"u{o}_{g}") for g in range(NG5)]
                  for o in range(2)]

            # W_ext(blk): rows 0:64 = Pdst_blk + b1m, rows 64:128 = W1m_edge
            for nb in range(NBLK):
                ps = psW.tile([64, H], f32, tag="psW")
                nc.tensor.matmul(
                    out=ps[:], lhsT=nfs_s[:, nb * BLK:(nb + 1) * BLK],
                    rhs=wdst_s[:], start=True, stop=False,
                )
                nc.tensor.matmul(
                    out=ps[:], lhsT=onesr_s[:, 0:BLK],
                    rhs=b1mr_s[:], start=False, stop=True,
                )
                nc.scalar.copy(out=wext[0:64, nb * H:(nb + 1) * H], in_=ps[:])
                nc.vector.tensor_copy(out=wext[64:128, nb * H:(nb + 1) * H],
                                      in_=w1e_s[:])

            # ---- stage 3 (emitted per 512-node group as soon as its eight
            # 64-node blocks have aggregated) ----
            def _stage3_group(g5):
                sl = slice(g5 * 512, (g5 + 1) * 512)
                for o in range(2):
                    osl = slice(o * 128, (o + 1) * 128)
                    # aggfinal = aggT.T@W2m + deg*b2m   (feature-major out)
                    pa = psS.tile([128, 512], f32, tag="psS", name=f"pa{g5}_{o}")
                    nc.tensor.matmul(out=pa[:], lhsT=w2m_s[:, 0 * H + o * 128:0 * H + (o + 1) * 128],
                                     rhs=aggT[0][g5][:], start=True, stop=False)
                    nc.tensor.matmul(out=pa[:], lhsT=w2m_s[:, 1 * H + o * 128:1 * H + (o + 1) * 128],
                                     rhs=aggT[1][g5][:], start=False, stop=False)
                    nc.tensor.matmul(out=pa[:], lhsT=b2mr_s[:, osl],
                                     rhs=degrow_s[:, sl], start=False, stop=True)
                    nc.vector.tensor_copy(out=agfT[o][g5][:], in_=pa[:])
                for o in range(2):
                    # u = gelu(concat(x, aggfinal) @ W1u + b1u)
                    pu = psS.tile([128, 512], f32, tag="psS", name=f"pu{g5}_{o}")
                    nc.tensor.matmul(out=pu[:], lhsT=w1u_s[:, 0 * H + o * 128:0 * H + (o + 1) * 128],
                                     rhs=nfs_s[:, sl], start=True, stop=False)
                    nc.tensor.matmul(out=pu[:], lhsT=w1u_s[:, 1 * H + o * 128:1 * H + (o + 1) * 128],
                                     rhs=agfT[0][g5][:], start=False, stop=False)
                    nc.tensor.matmul(out=pu[:], lhsT=w1u_s[:, 2 * H + o * 128:2 * H + (o + 1) * 128],
                                     rhs=agfT[1][g5][:], start=False, stop=True)
                    nc.scalar.activation(
                        out=uT[o][g5][:], in_=pu[:],
                        func=mybir.ActivationFunctionType.Gelu_apprx_tanh,
                        bias=b1uc_s[:, o:o + 1],
                    )
                # out = u @ W2u + b2u   (token-major out per node block)
                for j5 in range(4):
                    nb = g5 * 4 + j5
                    csl = slice(nb * 128, (nb + 1) * 128)
                    jsl = slice(j5 * 128, (j5 + 1) * 128)
                    po = psS.tile([128, 128], f32, tag="psS", name=f"po{g5}_{j5}")
                    nc.tensor.matmul(out=po[:], lhsT=uT[0][g5][:, jsl], rhs=w2u_s[:, 0:D],
                                     start=True, stop=False)
                    nc.tensor.matmul(out=po[:], lhsT=uT[1][g5][:, jsl], rhs=w2u_s[:, D:2 * D],
                                     start=False, stop=False)
                    nc.tensor.matmul(out=po[:], lhsT=onesr_s[:], rhs=b2ur_s[:],
                                     start=False, stop=True)
                    oc = cp.tile([128, 128], f32, tag="ocp")
                    nc.vector.tensor_copy(out=oc[:], in_=po[:])
                    nc.sync.dma_start(out=out[csl, :], in_=oc[:])

            # ---- stage 2: edge pipeline ----
            GCH = 4  # chunks per compute group (PSUM-bank limited)
            qn = 0
            for blk in range(NBLK):
                c0 = blk * k_blk * 128
                # fused lhsT tile: rows 0:64 = onehot64(dst_rel),
                # rows 64:128 = edge features (host-built, fp8)
                et = eftp.tile([128, k_blk * 128], fp8, tag="eh")
                nc.sync.dma_start(out=et[:], in_=ehot[:, c0:c0 + k_blk * 128])
                # scatter selection matrices (host-built, fp8, 0/1 exact)
                selB = selp.tile([128, k_blk, BLK], fp8, tag="sel")
                nc.sync.dma_start(
                    out=selB[:],
                    in_=selb[:, blk * k_blk * BLK:(blk + 1) * k_blk * BLK]
                        .rearrange("p (a n) -> p a n", n=BLK),
                )

                # gathers: one per (block, src-half); pads use idx=-1 (skipped)
                if blk < 4:
                    gt = gts_pre[blk]
                else:
                    gt = gpool.tile([128, k_blk, H], fp8, tag="gath")
                for hi, (g0, g1, pall_h) in enumerate(
                        ((0, k0, pall0), (k0, k_blk, pall1))):
                    gw = g1 - g0
                    cg = blk * k_blk + g0
                    nc.gpsimd.dma_gather(
                        gt[:, g0:g1, :],
                        pall_h[:],
                        gidx_s[:, cg * 8:cg * 8 + gw * 8],
                        num_idxs=gw * 128,
                        num_idxs_reg=vcnt[2 * blk + hi],
                        elem_size=H,
                        single_packet=False,
                        queue_num=qn,
                    )
                    qn = (qn + 1) % 4

                ag0t = psG.tile([128, BLK], f32, tag="agg")
                ag1t = psG.tile([128, BLK], f32, tag="agg")
                ag0 = ag0t[:]
                ag1 = ag1t[:]
                wext_blk = wext[:, blk * H:(blk + 1) * H]
                for g0 in range(0, k_blk, GCH):
                    g1 = min(g0 + GCH, k_blk)
                    gw = g1 - g0
                    # fused dense part (edge proj + Pdst + b1m) in PSUM
                    pe4 = psA.tile([128, GCH * H], f32, tag="psA")
                    for k in range(gw):
                        ck = g0 + k
                        nc.tensor.matmul(
                            out=pe4[:, k * H:(k + 1) * H],
                            lhsT=et[:, ck * 128:(ck + 1) * 128],
                            rhs=wext_blk, start=True, stop=True,
                        )
                    # s = Psrc[src] + pe4 ; h = gelu(s)
                    s4 = hp.tile([128, GCH * H], bf16, tag="s")
                    nc.vector.tensor_add(
                        out=s4[:, 0:gw * H],
                        in0=gt[:, g0:g1, :].rearrange("p a n -> p (a n)"),
                        in1=pe4[:, 0:gw * H],
                    )
                    h4 = hp.tile([128, GCH * H], bf16, tag="h")
                    nc.scalar.activation(
                        out=h4[:, 0:gw * H], in_=s4[:, 0:gw * H],
                        func=mybir.ActivationFunctionType.Gelu_apprx_tanh,
                    )
                    # scatter (feature-major): aggT[:, n] += h.T @ sel
                    for k in range(gw):
                        ck = g0 + k
                        nc.tensor.matmul(
                            out=ag0, lhsT=h4[:, k * H:k * H + 128],
                            rhs=selB[:, ck, :],
                            start=(ck == 0), stop=(ck == k_blk - 1),
                        )
                        nc.tensor.matmul(
                            out=ag1, lhsT=h4[:, k * H + 128:(k + 1) * H],
                            rhs=selB[:, ck, :],
                            start=(ck == 0), stop=(ck == k_blk - 1),
                        )
                g5, j5 = blk // BPG, blk % BPG
                csl = slice(j5 * BLK, (j5 + 1) * BLK)
                nc.vector.tensor_copy(out=aggT[0][g5][:, csl], in_=ag0)
                nc.scalar.copy(out=aggT[1][g5][:, csl], in_=ag1)
                if j5 == BPG - 1:
                    _stage3_group(g5)

    nc.finalize()
    return nc


def _prep_core_inputs(g, r, node_features, edge_indices, edge_features, kk, shared):
    """Host-side shard prep for core (graph g, dst-range r)."""
    k0, k1, _vc = kk
    k_blk = k0 + k1
    nchunk = NBLK * k_blk
    ecap = nchunk * 128
    dst = edge_indices[g, :, 1]
    src = edge_indices[g, :, 0]
    lo, hi = r * NSLICE, (r + 1) * NSLICE

    mask = (dst >= lo) & (dst < hi)
    eid = np.nonzero(mask)[0]
    dloc = dst[eid] - lo
    blk_of = dloc // BLK
    half_of = (src[eid] >= NPAD // 2).astype(np.int64)
    order = np.lexsort((half_of, blk_of))
    eid = eid[order]
    dloc = dloc[order]
    blk_of = blk_of[order]
    half_of = half_of[order]

    # slot layout per block: [0, k0*128) = src-half-0 edges, then half-1;
    # pads get gather idx -1 (descriptor skipped) and drel -1 (sel all-zero)
    slot = np.zeros(ecap, dtype=np.int64) - 1
    srcpad = np.full(ecap, -1, dtype=np.int64)
    drel = np.full(ecap, -1.0, dtype=np.float64)
    vcnt = shared["_vcnt"]
    for b in range(NBLK):
        for h in (0, 1):
            ids = eid[(blk_of == b) & (half_of == h)]
            cnt = len(ids)
            s0 = b * k_blk * 128 + (0 if h == 0 else k0 * 128)
            srcpad[s0:s0 + cnt] = src[ids] - h * (NPAD // 2)
            srcpad[s0 + cnt:s0 + vcnt[2 * b + h]] = 0
            dl = dst[ids] - lo
            drel[s0:s0 + cnt] = (dl - b * BLK).astype(np.float64)
            slot[s0:s0 + cnt] = ids

    # fused lhsT (fp8): rows 0:64 onehot64(dst_rel), rows 64:128 edge features
    valid = slot >= 0
    ehotc = np.zeros((128, ecap), dtype=FP8)
    vidx = np.nonzero(valid)[0]
    ehotc[drel[vidx].astype(np.int64), vidx] = FP8(1.0)
    ehotc[64:128, valid] = edge_features[g, slot[valid], :].T.astype(FP8)

    # scatter one-hot (fp8): selbc[p, c, n] = (drel[slot c*128+p] == n)
    dmat = drel.reshape(nchunk, 128).T  # [128, nchunk]
    selbc = (dmat[:, :, None] == np.arange(BLK, dtype=np.float64)[None, None, :])
    selbc = np.ascontiguousarray(selbc.reshape(128, nchunk * BLK)).astype(FP8)

    # gather indices (src only); wrapped in 16 partitions, replicated per Q7 core
    gidxc = np.tile(srcpad.astype(np.int16).reshape(-1, 16).T, (8, 1))

    deg = np.bincount(dloc, minlength=NSLICE).astype(np.float64)
    degc = deg[None, :].astype(BF16)

    inp = dict(shared)
    inp["nft"] = shared["_nftg"][g]
    inp["nfs"] = np.ascontiguousarray(shared["_nftg"][g][:, lo:hi])
    inp["ehot"] = ehotc
    inp["selb"] = selbc
    inp["gidx"] = gidxc
    inp["degrow"] = degc
    return {k: v for k, v in inp.items() if not k.startswith("_")}


def kernel(node_features, edge_indices, edge_features,
           W1m, b1m, W2m, b2m, W1u, b1u, W2u, b2u):
    node_features = np.asarray(node_features)
    edge_indices = np.asarray(edge_indices)
    edge_features = np.asarray(edge_features)

    # per-half chunks per node block: driven by max (block, src-half) occupancy
    dst = edge_indices[..., 1]
    srcv = edge_indices[..., 0]
    blk_id = 2 * ((np.arange(B)[:, None] * (NPAD // BLK)) + dst // BLK) \
        + (srcv >= NPAD // 2)
    counts = np.bincount(blk_id.reshape(-1), minlength=2 * B * NPAD // BLK)
    k0 = int(np.ceil(counts[0::2].max() / 128.0))
    k1 = int(np.ceil(counts[1::2].max() / 128.0))
    # static gather valid-counts: per (block-in-core, half) max over the 8 cores
    cmat = counts.reshape(B, CPG, NBLK, 2)
    vcnt = tuple(int(v) for v in cmat.max(axis=(0, 1)).reshape(-1))
    kk = (k0, k1, vcnt)

    if kk not in _BUILD_CACHE:
        _BUILD_CACHE[kk] = _build(k0, k1, vcnt)
    nc = _BUILD_CACHE[kk]

    # node features transposed + padded, bf16, per graph
    nftg = np.zeros((B, D, NPAD), dtype=BF16)
    for g in range(B):
        nftg[g, :, :N] = np.asarray(node_features[g]).T.astype(BF16)

    shared = {
        "_nftg": nftg,
        "_vcnt": vcnt,
        "w1e": np.asarray(W1m)[2 * D:].astype(BF16),
        "b1mr": np.asarray(b1m)[None, :].astype(BF16),
        "wsrc": np.asarray(W1m)[:D].astype(BF16),
        "wdst": np.asarray(W1m)[D:2 * D].astype(BF16),
        "w2m": np.asarray(W2m).reshape(2, 128, H).transpose(1, 0, 2).reshape(128, 2 * H).astype(BF16),
        "b2mr": np.asarray(b2m)[None, :].astype(BF16),
        "w1u": np.asarray(W1u).reshape(3, 128, H).transpose(1, 0, 2).reshape(128, 3 * H).astype(BF16),
        "b1uc": np.asarray(b1u).reshape(2, 128).T.astype(np.float32).copy(),
        "w2u": np.asarray(W2u).reshape(2, 128, D).transpose(1, 0, 2).reshape(128, 2 * D).astype(BF16),
        "b2ur": np.asarray(b2u)[None, :].astype(BF16),
        "onesr": np.ones((1, 128), dtype=BF16),
    }

    in_maps = []
    for c in range(NCORES):
        g, r = c // CPG, c % CPG
        in_maps.append(_prep_core_inputs(
            g, r, node_features, edge_indices, edge_features, kk, shared))

    global _LAST_IN_MAPS
    _LAST_IN_MAPS = in_maps
    res = run_bass_kernel_spmd(nc, in_maps, core_ids=list(range(NCORES)))

    outp = np.zeros((B, NPAD, D), dtype=np.float32)
    for c in range(NCORES):
        g, r = c // CPG, c % CPG
        outp[g, r * NSLICE:(r + 1) * NSLICE, :] = res.results[c]["out"]
    return outp[:, :N, :]
